# revision 10
# baseline (speedup 1.0000x reference)
import numpy as np

B = 8
SEQ = 4096
D = 1024
N_BASE = 10000.0
N_CORES = 8
SPC = SEQ // N_CORES  # seq rows per core
JT = SPC // 128       # 128-row chunks per core
G_DEFAULT = 2         # batches per DMA chunk (chunk = G*2MiB)
BUFS = 5

_CACHE = {}


def _compute_pe() -> np.ndarray:
    """Mirror of the reference _pos_encoding (default jax backend, f32)."""
    import jax
    import jax.numpy as jnp

    pos = jnp.arange(SEQ, dtype=jnp.float32)[:, None]
    i = jnp.arange(D // 2, dtype=jnp.float32)
    denom = jnp.power(jnp.float32(N_BASE), 2.0 * i / jnp.float32(D))
    ang = pos / denom
    pe = jnp.stack([jnp.sin(ang), jnp.cos(ang)], axis=-1).reshape(SEQ, D)
    return np.asarray(jax.device_get(pe), dtype=np.float32)


def _repack(x, c, G):
    xs = np.ascontiguousarray(x[:, c * SPC : (c + 1) * SPC, :])
    NG = B // G
    return np.ascontiguousarray(
        xs.reshape(NG, G, 128, JT, D).transpose(0, 2, 1, 3, 4)
    ).reshape(B * SPC, D)


def _unpack(y, G):
    NG = B // G
    return np.ascontiguousarray(
        y.reshape(NG, 128, G, JT, D).transpose(0, 2, 1, 3, 4)
    ).reshape(B, SPC, D)


def _build_program(G, bufs=BUFS):
    import concourse.bacc as bacc
    import concourse.mybir as mybir
    import concourse.tile as tile

    NG = B // G
    K = G * JT
    nc = bacc.Bacc("TRN2")
    f32 = mybir.dt.float32
    x_in = nc.declare_dram_parameter("x", [B * SPC, D], f32, isOutput=False)
    pe_in = nc.declare_dram_parameter("pe", [SPC, D], f32, isOutput=False)
    y_out = nc.declare_dram_parameter("y", [B * SPC, D], f32, isOutput=True)

    with tile.TileContext(nc) as tc:
        with (
            tc.tile_pool(name="pe_pool", bufs=1) as pe_pool,
            tc.tile_pool(name="x_pool", bufs=min(bufs, NG)) as x_pool,
        ):
            pe_t = pe_pool.tile([128, JT, D], f32)
            pe_ap = pe_in.rearrange("(p u) d -> p u d", u=JT)
            nc.sync.dma_start(out=pe_t[:], in_=pe_ap)
            for g in range(NG):
                xs = x_in[g * 128 * K : (g + 1) * 128 * K, :].rearrange(
                    "(p k) d -> p k d", k=K
                )
                xt = x_pool.tile([128, K, D], f32)
                nc.sync.dma_start(out=xt[:], in_=xs)
                for bb in range(G):
                    sl = xt[:, bb * JT : (bb + 1) * JT, :]
                    nc.vector.tensor_add(sl, sl, pe_t[:])
                ys = y_out[g * 128 * K : (g + 1) * 128 * K, :].rearrange(
                    "(p k) d -> p k d", k=K
                )
                nc.scalar.dma_start(out=ys, in_=xt[:])
    if not nc.is_finalized():
        nc.finalize()
    return nc


def _get_state(G=G_DEFAULT):
    if G not in _CACHE:
        _CACHE[G] = _build_program(G)
    if "pe" not in _CACHE:
        _CACHE["pe"] = _compute_pe()
    return _CACHE[G], _CACHE["pe"]


def kernel(x, seq_len=None, **_):
    from concourse.bass_utils import run_bass_kernel_spmd

    x = np.asarray(x, dtype=np.float32)
    assert x.shape == (B, SEQ, D)
    if seq_len is not None:
        assert int(np.asarray(seq_len)) == SEQ

    G = G_DEFAULT
    nc, pe = _get_state(G)
    in_maps = []
    for c in range(N_CORES):
        pes = np.ascontiguousarray(pe[c * SPC : (c + 1) * SPC, :])
        in_maps.append({"x": _repack(x, c, G), "pe": pes})

    res = run_bass_kernel_spmd(nc, in_maps, list(range(N_CORES))).results

    out = np.empty((B, SEQ, D), dtype=np.float32)
    for c in range(N_CORES):
        out[:, c * SPC : (c + 1) * SPC, :] = _unpack(res[c]["y"], G)
    return out


# revision 11
# speedup vs baseline: 1.0500x; 1.0500x over previous
import numpy as np

B = 8
SEQ = 4096
D = 1024
N_BASE = 10000.0
N_CORES = 8
SPC = SEQ // N_CORES  # seq rows per core
JT = SPC // 128       # 128-row chunks per core
G_DEFAULT = 4         # batches per DMA chunk (chunk = G*2MiB)
BUFS = 5

_CACHE = {}


def _compute_pe() -> np.ndarray:
    """Mirror of the reference _pos_encoding (default jax backend, f32)."""
    import jax
    import jax.numpy as jnp

    pos = jnp.arange(SEQ, dtype=jnp.float32)[:, None]
    i = jnp.arange(D // 2, dtype=jnp.float32)
    denom = jnp.power(jnp.float32(N_BASE), 2.0 * i / jnp.float32(D))
    ang = pos / denom
    pe = jnp.stack([jnp.sin(ang), jnp.cos(ang)], axis=-1).reshape(SEQ, D)
    return np.asarray(jax.device_get(pe), dtype=np.float32)


def _repack(x, c, G):
    xs = np.ascontiguousarray(x[:, c * SPC : (c + 1) * SPC, :])
    NG = B // G
    return np.ascontiguousarray(
        xs.reshape(NG, G, 128, JT, D).transpose(0, 2, 1, 3, 4)
    ).reshape(B * SPC, D)


def _unpack(y, G):
    NG = B // G
    return np.ascontiguousarray(
        y.reshape(NG, 128, G, JT, D).transpose(0, 2, 1, 3, 4)
    ).reshape(B, SPC, D)


def _build_program(G, bufs=BUFS):
    import concourse.bacc as bacc
    import concourse.mybir as mybir
    import concourse.tile as tile

    NG = B // G
    K = G * JT
    nc = bacc.Bacc("TRN2")
    f32 = mybir.dt.float32
    x_in = nc.declare_dram_parameter("x", [B * SPC, D], f32, isOutput=False)
    pe_in = nc.declare_dram_parameter("pe", [SPC, D], f32, isOutput=False)
    y_out = nc.declare_dram_parameter("y", [B * SPC, D], f32, isOutput=True)

    with tile.TileContext(nc) as tc:
        with (
            tc.tile_pool(name="pe_pool", bufs=1) as pe_pool,
            tc.tile_pool(name="x_pool", bufs=min(bufs, NG)) as x_pool,
        ):
            pe_t = pe_pool.tile([128, JT, D], f32)
            pe_ap = pe_in.rearrange("(p u) d -> p u d", u=JT)
            nc.sync.dma_start(out=pe_t[:], in_=pe_ap)
            for g in range(NG):
                xs = x_in[g * 128 * K : (g + 1) * 128 * K, :].rearrange(
                    "(p k) d -> p k d", k=K
                )
                xt = x_pool.tile([128, K, D], f32)
                nc.sync.dma_start(out=xt[:], in_=xs)
                ys = y_out[g * 128 * K : (g + 1) * 128 * K, :].rearrange(
                    "(p k) d -> p k d", k=K
                )
                for bb in range(G):
                    sl = xt[:, bb * JT : (bb + 1) * JT, :]
                    nc.vector.tensor_add(sl, sl, pe_t[:])
                    nc.scalar.dma_start(
                        out=ys[:, bb * JT : (bb + 1) * JT, :], in_=sl
                    )
    if not nc.is_finalized():
        nc.finalize()
    return nc


def _get_state(G=G_DEFAULT):
    if G not in _CACHE:
        _CACHE[G] = _build_program(G)
    if "pe" not in _CACHE:
        _CACHE["pe"] = _compute_pe()
    return _CACHE[G], _CACHE["pe"]


def kernel(x, seq_len=None, **_):
    from concourse.bass_utils import run_bass_kernel_spmd

    x = np.asarray(x, dtype=np.float32)
    assert x.shape == (B, SEQ, D)
    if seq_len is not None:
        assert int(np.asarray(seq_len)) == SEQ

    G = G_DEFAULT
    nc, pe = _get_state(G)
    in_maps = []
    for c in range(N_CORES):
        pes = np.ascontiguousarray(pe[c * SPC : (c + 1) * SPC, :])
        in_maps.append({"x": _repack(x, c, G), "pe": pes})

    res = run_bass_kernel_spmd(nc, in_maps, list(range(N_CORES))).results

    out = np.empty((B, SEQ, D), dtype=np.float32)
    for c in range(N_CORES):
        out[:, c * SPC : (c + 1) * SPC, :] = _unpack(res[c]["y"], G)
    return out
